# revision 27
# baseline (speedup 1.0000x reference)
"""Single-step LSTM cell (B=131072, E=H=128) on 8 Trainium2 NeuronCores.

Strategy: pure data-parallel over the batch; each core handles 16384 rows
in transposed layout (contraction dim on SBUF partitions, no on-chip
transposes). The ACT (scalar) engine is the hard floor — 5 transcendental
evals per element (sigmoid i/f/o, tanh(c~), tanh(c)) at 1 col/cycle — so
everything else is shaped to stay out of its way:

- Per-gate activation instructions with a per-partition bias AP fold the
  gate biases into the sigmoid/tanh, eliminating the bias matmuls (PE does
  only the 2 real GEMM accumulations per gate).
- The c~ gate uses a real Tanh (same ACT table set as Sigmoid), removing
  the 2*sigmoid(2z)-1 DVE fixup.
- c is bf16 end to end: halves the c HBM traffic (total ~20.3 MiB/core)
  and keeps every DVE op in 2x 16-bit mode.
- PSUM holds two [128,2048] fp32 gate tiles (8 banks) in a double-buffered
  rotation; PE bf16 fill rate (2 matmuls/col @ 2.4GHz) exactly matches
  ACT's 1 col/cycle drain, so the pipeline is rate-balanced and ACT (with
  its extra tanh(c) instruction per group) stays the limiter.
- Group sizes 1024, 2048x7, 1024: a small first group starts the ACT
  stream early, a small last group shortens the drain.
- All data loads ride ONE HWDGE ring (SP) in priority order — rings are
  FIFO, so group 0 lands first at full bandwidth. The tiny W/U/bias loads
  ride the ACT ring, dispatched BEFORE the table-load/dummy-sigmoid pair
  so they fire immediately at engine start. Mid-stream stores use the
  idle GPSIMD SWDGE ring (they never block load dispatches); the last
  group's stores use SP (lower latency) to shorten the tail.
- Warmup matmuls off a memset tile ramp the PE clock (HAM) and a dummy
  sigmoid pulls the ACT table load off the critical path.
"""

import numpy as np

B, E, H = 131072, 128, 128
NCORES = 8
BC = B // NCORES        # 16384 batch rows per core
G = 2048                # steady-state group size
MM = 512                # matmul free-dim tile (one PSUM bank)
SIZES = [1024] + [2048] * 7 + [1024]
assert sum(SIZES) == BC

_CACHE = {}


def _build_nc():
    import concourse.bacc as bacc
    import concourse.mybir as mybir
    import concourse.tile as tile

    f32 = mybir.dt.float32
    bf = mybir.dt.bfloat16
    AF = mybir.ActivationFunctionType
    ALU = mybir.AluOpType

    nc = bacc.Bacc("TRN2", target_bir_lowering=False, debug=False,
                   num_devices=NCORES)

    xT = nc.dram_tensor("xT", [E, BC], bf, kind="ExternalInput").ap()
    hT = nc.dram_tensor("hT", [H, BC], bf, kind="ExternalInput").ap()
    cT = nc.dram_tensor("cT", [H, BC], bf, kind="ExternalInput").ap()
    W = nc.dram_tensor("W", [E, 4 * H], bf, kind="ExternalInput").ap()
    U = nc.dram_tensor("U", [H, 4 * H], bf, kind="ExternalInput").ap()
    bias4 = nc.dram_tensor("bias4", [H, 4], f32, kind="ExternalInput").ap()
    hT_out = nc.dram_tensor("hT_out", [H, BC], bf, kind="ExternalOutput").ap()
    cT_out = nc.dram_tensor("cT_out", [H, BC], bf, kind="ExternalOutput").ap()

    NG = len(SIZES)
    OFFS = [sum(SIZES[:g]) for g in range(NG)]

    with tile.TileContext(nc) as tc:
        with tc.tile_pool(name="cst", bufs=1) as cst, \
             tc.tile_pool(name="xin", bufs=3) as xin, \
             tc.tile_pool(name="hin", bufs=3) as hin, \
             tc.tile_pool(name="cin", bufs=3) as cin, \
             tc.tile_pool(name="sig", bufs=2) as sigp, \
             tc.tile_pool(name="tcp", bufs=2) as tcp, \
             tc.tile_pool(name="cop", bufs=2) as cop, \
             tc.tile_pool(name="hop", bufs=2) as hop, \
             tc.tile_pool(name="ps", bufs=2, space="PSUM") as ps:

            W_sb = cst.tile([E, 4 * H], bf)
            U_sb = cst.tile([H, 4 * H], bf)
            b_sb = cst.tile([H, 4], f32)

            # warmup source + ACT table preload (no DMA dependencies)
            dum = cst.tile([H, 16], bf, name="dum")
            dumo = cst.tile([H, 16], bf, name="dumo")
            wsrc = cst.tile([E, MM], bf, name="wsrc")
            nc.vector.memset(dum[:], 0.0)
            nc.vector.memset(wsrc[:], 1.0)
            nc.scalar.activation(dumo[:], dum[:], AF.Sigmoid)

            def load_group(g):
                gsz, off = SIZES[g], OFFS[g]
                xg = xin.tile([E, G], bf, tag="x")
                hg = hin.tile([H, G], bf, tag="h")
                cg = cin.tile([H, G], bf, tag="c")
                nc.sync.dma_start(out=xg[:, 0:gsz], in_=xT[:, off:off + gsz])
                nc.sync.dma_start(out=hg[:, 0:gsz], in_=hT[:, off:off + gsz])
                nc.sync.dma_start(out=cg[:, 0:gsz], in_=cT[:, off:off + gsz])
                return xg, hg, cg

            # consts on the ACT ring (parallel with the SP ring, tiny);
            # data groups on the SP ring in priority order
            nc.scalar.dma_start(out=W_sb[:], in_=W)
            nc.scalar.dma_start(out=U_sb[:], in_=U)
            nc.scalar.dma_start(out=b_sb[:], in_=bias4)
            tiles = {0: load_group(0), 1: load_group(1)}

            # PE clock (HAM) warmup while the first chunk loads
            warm = ps.tile([H, G], f32, name="warm", tag="ps")
            for _ in range(6):
                nc.tensor.matmul(warm[:, 0:MM], wsrc[:, 0:H], wsrc[:],
                                 start=True, stop=True)

            pend = None         # (o_t, co, off, gsz) waiting for tanh(c)
            GATE_K = {"i": 0, "f": 1, "o": 2, "ct": 3}

            def gate(tg, x_sb, h_sb, gsz):
                """matmul-accumulate one gate into PSUM, activate to SBUF"""
                k = GATE_K[tg]
                fn = AF.Tanh if tg == "ct" else AF.Sigmoid
                gt = ps.tile([H, G], f32, tag="ps")
                Wg = W_sb[:, k * H:(k + 1) * H]
                Ug = U_sb[:, k * H:(k + 1) * H]
                for s in range(0, gsz, MM):
                    nc.tensor.matmul(gt[:, s:s + MM], Wg,
                                     x_sb[:, s:s + MM],
                                     start=True, stop=False)
                    nc.tensor.matmul(gt[:, s:s + MM], Ug,
                                     h_sb[:, s:s + MM],
                                     start=False, stop=True)
                st = sigp.tile([H, G], bf, tag=tg, bufs=2)
                nc.scalar.activation(st[:, 0:gsz], gt[:, 0:gsz], fn,
                                     bias=b_sb[:, k:k + 1])
                return st

            def do_pend(store_q=None, tok=None):
                # tok (a [128,1] tile holding exactly 1.0, derived from the
                # CURRENT group's m2) forces the scheduler to place this
                # tanh(c) at the END of the current group's ACT sequence:
                # the PSUM buffer that the next group's i-gate reuses then
                # frees two ACT instructions before the group boundary, so
                # the PE refill always beats ACT there (no boundary gap).
                po, pco, poff, pgsz = pend
                tc_sb = tcp.tile([H, G], bf, tag="tc")
                scale = tok[:, 0:1] if tok is not None else 1.0
                nc.scalar.activation(tc_sb[:, 0:pgsz], pco[:, 0:pgsz],
                                     AF.Tanh, scale=scale)
                ho_sb = hop.tile([H, G], bf, tag="ho")
                nc.vector.tensor_mul(out=ho_sb[:, 0:pgsz],
                                     in0=po[:, 0:pgsz],
                                     in1=tc_sb[:, 0:pgsz])
                (store_q or nc.gpsimd).dma_start(
                    out=hT_out[:, poff:poff + pgsz], in_=ho_sb[:, 0:pgsz])

            for g in range(NG - 1):
                gsz, off = SIZES[g], OFFS[g]
                x_sb, h_sb, c_sb = tiles.pop(g)
                if g + 2 < NG:
                    tiles[g + 2] = load_group(g + 2)

                # gates: i, f, o, c~ — gate-major so ACT can drain gate k
                # while PE fills gate k+1 (2 PSUM tiles = 8 banks total)
                i_t = gate("i", x_sb, h_sb, gsz)
                f_t = gate("f", x_sb, h_sb, gsz)
                o_t = gate("o", x_sb, h_sb, gsz)
                if pend is not None:
                    # scale token off o_t: ready during this group's last
                    # gate activation, so the pinned tanh(c) below incurs
                    # no extra wait at the group boundary
                    tok = cop.tile([H, 1], f32, tag="tok", bufs=2)
                    nc.vector.tensor_scalar(out=tok[:], in0=o_t[:, 0:1],
                                            scalar1=0.0, scalar2=1.0,
                                            op0=ALU.mult, op1=ALU.add)
                ct_t = gate("ct", x_sb, h_sb, gsz)

                # c = f*c_prev + i*c~  (all bf16, DVE 2x mode)
                m1 = cop.tile([H, G], bf, tag="m1", bufs=2)
                m2 = cop.tile([H, G], bf, tag="m2", bufs=2)
                nc.vector.tensor_mul(out=m1[:, 0:gsz], in0=f_t[:, 0:gsz],
                                     in1=c_sb[:, 0:gsz])
                nc.vector.tensor_mul(out=m2[:, 0:gsz], in0=i_t[:, 0:gsz],
                                     in1=ct_t[:, 0:gsz])

                # tanh(c) of the previous group, pinned to the end of this
                # group's ACT sequence via the scale token
                if pend is not None:
                    do_pend(tok=tok)

                co_sb = cop.tile([H, G], bf, tag="co", bufs=2)
                nc.vector.tensor_add(out=co_sb[:, 0:gsz], in0=m1[:, 0:gsz],
                                     in1=m2[:, 0:gsz])
                nc.gpsimd.dma_start(out=cT_out[:, off:off + gsz],
                                    in_=co_sb[:, 0:gsz])

                pend = (o_t, co_sb, off, gsz)

            # last group (1024 cols): c~ and i first so the DVE c-path
            # overlaps the remaining gates; SP stores (lower latency than
            # SWDGE) shorten the drain
            g = NG - 1
            gsz, off = SIZES[g], OFFS[g]
            x_sb, h_sb, c_sb = tiles.pop(g)
            ct_t = gate("ct", x_sb, h_sb, gsz)
            i_t = gate("i", x_sb, h_sb, gsz)
            m2 = cop.tile([H, G], bf, tag="m2", bufs=2)
            nc.vector.tensor_mul(out=m2[:, 0:gsz], in0=i_t[:, 0:gsz],
                                 in1=ct_t[:, 0:gsz])
            f_t = gate("f", x_sb, h_sb, gsz)
            m1 = cop.tile([H, G], bf, tag="m1", bufs=2)
            co_sb = cop.tile([H, G], bf, tag="co", bufs=2)
            nc.vector.tensor_mul(out=m1[:, 0:gsz], in0=f_t[:, 0:gsz],
                                 in1=c_sb[:, 0:gsz])
            nc.vector.tensor_add(out=co_sb[:, 0:gsz], in0=m1[:, 0:gsz],
                                 in1=m2[:, 0:gsz])
            nc.sync.dma_start(out=cT_out[:, off:off + gsz],
                              in_=co_sb[:, 0:gsz])
            do_pend()
            o_t = gate("o", x_sb, h_sb, gsz)
            tc_sb = tcp.tile([H, G], bf, tag="tc")
            nc.scalar.activation(tc_sb[:, 0:gsz], co_sb[:, 0:gsz], AF.Tanh)
            ho_sb = hop.tile([H, G], bf, tag="ho")
            nc.vector.tensor_mul(out=ho_sb[:, 0:gsz], in0=o_t[:, 0:gsz],
                                 in1=tc_sb[:, 0:gsz])
            nc.sync.dma_start(out=hT_out[:, off:off + gsz],
                              in_=ho_sb[:, 0:gsz])

    nc.compile()
    return nc


def kernel(x, hidden_memory_tm1, Wi, Ui, bi, Wf, Uf, bf, Wog, Uog, bog,
           Wc, Uc, bc, _return_timing=False, _trace=False):
    from concourse.bass_utils import run_bass_kernel_spmd

    if "nc" not in _CACHE:
        _CACHE["nc"] = _build_nc()
    nc = _CACHE["nc"]

    import ml_dtypes
    bf16 = ml_dtypes.bfloat16
    x = np.asarray(x, np.float32)
    hm = np.asarray(hidden_memory_tm1, np.float32)
    W = np.concatenate([Wi, Wf, Wog, Wc], axis=1).astype(bf16)
    U = np.concatenate([Ui, Uf, Uog, Uc], axis=1).astype(bf16)
    bias4 = np.stack([np.asarray(bi), np.asarray(bf), np.asarray(bog),
                      np.asarray(bc)], axis=1).astype(np.float32)

    in_maps = []
    for c in range(NCORES):
        sl = slice(c * BC, (c + 1) * BC)
        in_maps.append({
            "xT": np.ascontiguousarray(x[sl].astype(bf16).T),
            "hT": np.ascontiguousarray(hm[0, sl].astype(bf16).T),
            "cT": np.ascontiguousarray(hm[1, sl].astype(bf16).T),
            "W": W, "U": U, "bias4": bias4,
        })

    res = run_bass_kernel_spmd(nc, in_maps, core_ids=list(range(NCORES)),
                               trace=_trace)

    h = np.concatenate(
        [res.results[c]["hT_out"].T.astype(np.float32) for c in range(NCORES)], 0)
    cc = np.concatenate(
        [res.results[c]["cT_out"].T.astype(np.float32) for c in range(NCORES)], 0)
    out = np.stack([h, cc])
    if _return_timing:
        return out, res
    return out
